# revision 17
# baseline (speedup 1.0000x reference)
"""GNN NodeBlock kernel for Trainium2, 8 NeuronCores (SPMD, no collectives).

Reference computation (N=50000 nodes, E=1600000 edges, F=128 features):
    recv_agg = segment_sum(edge_attr, edge_index[1], N)        # [N, 128]
    collected = concat([recv_agg, x, broadcast(u)], -1)        # [N, 272]
    out = relu(collected @ W1 + b1) @ W2 + b2                  # [N, 128]

Sharding: nodes are assigned to 8 cores x 196 tiles of 32 nodes by a
degree-balancing permutation (serpentine deal over degree-sorted nodes +
swap repair), so every tile owns <= 1024 edges and the edge buffer packs
into a uniform 8 chunks of 128 edges per tile with ~0.4% padding.  Edges
are bucketed by receiver-node ownership so the scatter-sum is local;
MLP weights are replicated; the u-term of layer 1 is folded into b1.

Device algorithm per core (196 tiles = 49 groups of 128 nodes):
  scatter: one DVE is_equal per supertile builds one-hot routing blocks
    onehot[e, q, n] = (rel[e, q] == n); for each 128-node group the PE
    runs 8 accumulation steps of 4 column-tiled matmuls (strip j =
    tile 4g+j, stationary = 32-col one-hot, moving = 128-col edge chunk)
    so all four 32x128 array strips compute concurrently;
    agg [n, f] is then PE-transposed to aggT [f, n].
  L1: h[h, n] = W1a[d, h]^T @ aggT[d, n] + W1b[d, h]^T @ xT[d, n],
    ReLU+bias on the scalar engine.
  L2: yT[f, n] = sum_hc W2r[h, f]^T @ hT[h, n]  (stationary = W2 chunk,
    streaming = hT), b2 added on the vector engine during PSUM evac.
  Output is stored feature-major [128, nodes]; the host transposes and
  un-permutes.
All matmul inputs are bf16 (fp32 PSUM accumulation); one-hots are exact.
"""

import numpy as np

from concourse import bacc, mybir, tile
from concourse import bass_utils
from concourse.bass_interp import get_hw_module

# ---------------- problem constants (hardcoded per spec) ----------------
N_NODES = 50000
N_EDGES = 1600000
F = 128           # edge/node feature dim
H = 1024          # hidden dim
HC = H // 128     # 8 hidden chunks
D_U = 16
N_CORES = 8
TN = 32                                # nodes per scatter tile
NT = 196                               # tiles per core
CPT = 8                                # chunks (of 128 edges) per tile
QT = NT * CPT                          # 1568 chunks per core
NODES_PAD = NT * TN                    # 6272 node slots per core
T_ALL = N_CORES * NT                   # 1568 tiles total
# supertile sizes (in tiles): small leading supers shorten the pipeline
# ramp; 16-tile (512-node) supers amortize matmul/LDW/ACT overheads
SUPERS = [4, 4, 8] + [16] * 11 + [4]
assert sum(SUPERS) == NT

EDGE_DT = mybir.dt.bfloat16
MLP_DT = mybir.dt.bfloat16
OUT_DT = mybir.dt.bfloat16             # on-device output store dtype

_np = mybir.dt.np  # mybir dtype -> numpy dtype


# ---------------- device program ----------------

def build_program():
    f32 = mybir.dt.float32

    nc = bacc.Bacc("TRN2", target_bir_lowering=False, debug=False,
                   num_devices=N_CORES)

    edges = nc.dram_tensor("edges", [128, QT, F], EDGE_DT,
                           kind="ExternalInput").ap()
    relT = nc.dram_tensor("relT", [128, QT], EDGE_DT,
                          kind="ExternalInput").ap()
    iota = nc.dram_tensor("iota", [128, TN], EDGE_DT,
                          kind="ExternalInput").ap()
    ident = nc.dram_tensor("ident", [128, 128], MLP_DT,
                           kind="ExternalInput").ap()
    xT = nc.dram_tensor("xT", [128, NODES_PAD], MLP_DT,
                        kind="ExternalInput").ap()
    w1a = nc.dram_tensor("w1a", [128, H], MLP_DT, kind="ExternalInput").ap()
    w1b = nc.dram_tensor("w1b", [128, H], MLP_DT, kind="ExternalInput").ap()
    w2r = nc.dram_tensor("w2r", [128, H], MLP_DT, kind="ExternalInput").ap()
    b1T = nc.dram_tensor("b1T", [128, HC], f32, kind="ExternalInput").ap()
    b2c = nc.dram_tensor("b2c", [128, 1], f32, kind="ExternalInput").ap()
    y = nc.dram_tensor("y", [128, NODES_PAD], OUT_DT,
                       kind="ExternalOutput").ap()

    with tile.TileContext(nc) as tc:
        with (
            tc.tile_pool(name="const", bufs=1) as cpool,
            tc.tile_pool(name="edge", bufs=6) as epool,
            tc.tile_pool(name="oh", bufs=6) as ohpool,
            tc.tile_pool(name="nf", bufs=9) as nfpool,
            tc.tile_pool(name="aggT", bufs=2) as aggpool,
            tc.tile_pool(name="h", bufs=18) as hpool,
            tc.tile_pool(name="out", bufs=2) as outpool,
            tc.tile_pool(name="ps_agg", bufs=2, space="PSUM") as ps_agg,
            tc.tile_pool(name="ps_tr", bufs=2, space="PSUM") as ps_tr,
            tc.tile_pool(name="ps_h", bufs=2, space="PSUM") as ps_h,
            tc.tile_pool(name="ps_o", bufs=2, space="PSUM") as ps_o,
        ):
            # relT/iota gate the first one-hot: load them on the sync ring
            # AHEAD of the edge stream; bulk constants go on the scalar ring
            relT_sb = cpool.tile([128, QT], EDGE_DT, tag="relT")
            nc.sync.dma_start(relT_sb[:], relT[:])
            iota_sb = cpool.tile([128, TN], EDGE_DT, tag="iota")
            nc.sync.dma_start(iota_sb[:], iota[:])
            ident_sb = cpool.tile([128, 128], MLP_DT, tag="ident")
            nc.scalar.dma_start(ident_sb[:], ident[:])
            xT_sb = cpool.tile([128, NODES_PAD], MLP_DT, tag="xT")
            nc.scalar.dma_start(xT_sb[:], xT[:])
            w1a_sb = cpool.tile([128, H], MLP_DT, tag="w1a")
            nc.scalar.dma_start(w1a_sb[:], w1a[:])
            w1b_sb = cpool.tile([128, H], MLP_DT, tag="w1b")
            nc.scalar.dma_start(w1b_sb[:], w1b[:])
            w2r_sb = cpool.tile([128, H], MLP_DT, tag="w2r")
            nc.scalar.dma_start(w2r_sb[:], w2r[:])
            b1T_sb = cpool.tile([128, HC], f32, tag="b1T")
            nc.scalar.dma_start(b1T_sb[:], b1T[:])
            b2c_sb = cpool.tile([128, 1], f32, tag="b2c")
            nc.scalar.dma_start(b2c_sb[:], b2c[:])

            def quad(ps_out, wsb, col0, rhs_ap, start, stop):
                """One 128-col matmul issued as 4 concurrent 32-col strips
                so LDWEIGHTS hides in per-subarray concurrency."""
                for j in range(4):
                    nc.tensor.matmul(
                        ps_out[j * 32:(j + 1) * 32, :],
                        lhsT=wsb[:, col0 + j * 32:col0 + (j + 1) * 32],
                        rhs=rhs_ap,
                        start=start, stop=stop,
                        skip_group_check=True,
                        tile_position=(0, j * 32),
                    )

            def flush_l2(prev, o_ps):
                """Evacuate + store the finished previous supertile's yT."""
                t0p, nnp, _ = prev
                y_sb = outpool.tile([128, nnp], OUT_DT, tag="y")
                b2_bc = b2c_sb[:].broadcast_to([128, nnp])
                nc.vector.tensor_tensor(out=y_sb[:], in0=o_ps[:], in1=b2_bc,
                                        op=mybir.AluOpType.add)
                nc.scalar.dma_start(y[:, t0p * TN:t0p * TN + nnp], y_sb[:])

            def do_scatter(t0s, nts):
                """Scatter a supertile in 128-node groups: per-group edge
                DMA (1 MB) + one-hot + 8 accumulation steps of 4
                column-tiled strips.  Fine granularity keeps PE idle
                slices under the HAM re-throttle window."""
                nn = nts * TN
                q0 = t0s * CPT
                agg_nfs = []
                for g in range(nn // 128):
                    gq0 = q0 + g * 4 * CPT     # 32 chunks per group
                    e_g = epool.tile([128, 4 * CPT, F], EDGE_DT, tag="e")
                    nc.sync.dma_start(e_g[:], edges[:, gq0:gq0 + 4 * CPT])
                    oh = ohpool.tile([128, 4 * CPT, TN], EDGE_DT, tag="oh")
                    rel_bc = relT_sb[:, gq0:gq0 + 4 * CPT].rearrange(
                        "p (c u) -> p c u", u=1).broadcast_to(
                            [128, 4 * CPT, TN])
                    iota_bc = iota_sb[:].rearrange(
                        "p (u n) -> p u n", u=1).broadcast_to(
                            [128, 4 * CPT, TN])
                    nc.vector.tensor_tensor(out=oh[:], in0=iota_bc,
                                            in1=rel_bc,
                                            op=mybir.AluOpType.is_equal)
                    agg_ps = ps_agg.tile([128, 128], f32, tag="agg")
                    for c in range(CPT):
                        for j in range(4):
                            q = j * CPT + c
                            nc.tensor.matmul(
                                agg_ps[j * TN:(j + 1) * TN, :],
                                lhsT=oh[:, q, :],
                                rhs=e_g[:, q, :],
                                start=(c == 0),
                                stop=(c == CPT - 1),
                                skip_group_check=True,
                                tile_position=(0, j * TN),
                            )
                    agg_nf = nfpool.tile([128, 128], MLP_DT, tag="nf")
                    nc.scalar.copy(agg_nf[:], agg_ps[:])
                    agg_nfs.append(agg_nf)
                return (t0s, nn, agg_nfs)

            def do_tr_l1(sc, l2prev):
                """Transpose + L1 for a scattered supertile (one super
                behind the scatter), interleaved with L2 of the super
                before that — every PE op here depends only on data
                finished at least a supertile ago, so the in-order PE
                queue never stalls on the scalar/vector engines."""
                t0s, nn, agg_nfs = sc
                aggT = aggpool.tile([128, nn], MLP_DT, tag="aggT")
                for g, agg_nf in enumerate(agg_nfs):
                    tr_ps = ps_tr.tile([128, 128], MLP_DT, tag="tr")
                    nc.tensor.transpose(tr_ps[:], agg_nf[:], ident_sb[:])
                    nc.vector.tensor_copy(aggT[:, g * 128:(g + 1) * 128],
                                          tr_ps[:])
                o_ps = None
                if l2prev is not None:
                    o_ps = ps_o.tile([128, l2prev[1]], f32, tag="ops")
                hTs = []
                for hc in range(HC):
                    h_ps = ps_h.tile([128, nn], f32, tag="h")
                    quad(h_ps, w1a_sb, hc * 128, aggT[:],
                         start=True, stop=False)
                    quad(h_ps, w1b_sb, hc * 128,
                         xT_sb[:, t0s * TN:t0s * TN + nn],
                         start=False, stop=True)
                    if l2prev is not None:
                        quad(o_ps, w2r_sb, hc * 128, l2prev[2][hc][:],
                             start=(hc == 0), stop=(hc == HC - 1))
                    hT = hpool.tile([128, nn], MLP_DT, tag="hT")
                    nc.scalar.activation(hT[:], h_ps[:],
                                         mybir.ActivationFunctionType.Relu,
                                         bias=b1T_sb[:, hc:hc + 1], scale=1.0)
                    hTs.append(hT)
                if l2prev is not None:
                    flush_l2(l2prev, o_ps)
                return (t0s, nn, hTs)

            t0 = 0
            sc_prev = None       # scattered, awaiting transpose+L1
            l1_prev = None       # L1 done, awaiting L2
            for s, nts in enumerate(SUPERS):
                sc = do_scatter(t0, nts)
                if sc_prev is not None:
                    l1_prev = do_tr_l1(sc_prev, l1_prev)
                sc_prev = sc
                t0 += nts
            l1_prev = do_tr_l1(sc_prev, l1_prev)
            # epilogue: layer 2 of the last supertile
            o_ps = ps_o.tile([128, l1_prev[1]], f32, tag="ops")
            for hc in range(HC):
                quad(o_ps, w2r_sb, hc * 128, l1_prev[2][hc][:],
                     start=(hc == 0), stop=(hc == HC - 1))
            flush_l2(l1_prev, o_ps)

    nc.compile()
    nc.m = get_hw_module(nc.m)
    return nc


# ---------------- host-side sharding / packing ----------------

def _balance_nodes(deg):
    """Assign nodes to (tile, slot) so every tile of 32 nodes owns
    <= CPT*128 edges.  Serpentine deal over degree-sorted nodes, then a
    swap repair for the handful of tiles that land 1-2 edges over."""
    n = deg.shape[0]
    order = np.argsort(-deg, kind="stable")
    node_tile = np.empty(n, np.int64)
    node_slot = np.empty(n, np.int64)
    rounds = (n + T_ALL - 1) // T_ALL
    for r in range(rounds):
        blk = order[r * T_ALL:(r + 1) * T_ALL]
        tiles = np.arange(len(blk))
        if r % 2 == 1:
            tiles = T_ALL - 1 - tiles
        node_tile[blk] = tiles
        node_slot[blk] = r
    cap = CPT * 128
    loads = np.bincount(node_tile, weights=deg.astype(np.float64),
                        minlength=T_ALL).astype(np.int64)
    if loads.max() > cap:
        # per-tile node lists for the repair pass
        by_tile = [[] for _ in range(T_ALL)]
        for nd in range(n):
            by_tile[node_tile[nd]].append(nd)
        lo_tiles = list(np.argsort(loads)[:4 * (loads > cap).sum() + 8])
        for t_hi in np.where(loads > cap)[0]:
            while loads[t_hi] > cap:
                need = loads[t_hi] - cap
                swapped = False
                for a in sorted(by_tile[t_hi], key=lambda v: -deg[v]):
                    for t_lo in lo_tiles:
                        room = cap - loads[t_lo]
                        for b in by_tile[t_lo]:
                            d = deg[a] - deg[b]
                            if need <= d <= room:
                                sa, sb = node_slot[a], node_slot[b]
                                node_tile[a], node_tile[b] = t_lo, t_hi
                                node_slot[a], node_slot[b] = sb, sa
                                by_tile[t_hi].remove(a)
                                by_tile[t_lo].remove(b)
                                by_tile[t_lo].append(a)
                                by_tile[t_hi].append(b)
                                loads[t_hi] -= d
                                loads[t_lo] += d
                                swapped = True
                                break
                        if swapped:
                            break
                    if swapped:
                        break
                assert swapped, "balance repair failed"
    assert loads.max() <= cap
    return node_tile, node_slot


def prepare_inputs(x, edge_attr, u, W1, b1, W2, b2, edge_index):
    x = np.asarray(x, dtype=np.float32)
    edge_attr = np.asarray(edge_attr, dtype=np.float32)
    u = np.asarray(u, dtype=np.float32)
    W1 = np.asarray(W1, dtype=np.float32)
    b1 = np.asarray(b1, dtype=np.float32)
    W2 = np.asarray(W2, dtype=np.float32)
    b2 = np.asarray(b2, dtype=np.float32)
    recv = np.asarray(edge_index)[1].astype(np.int64)

    edge_np = _np(EDGE_DT)
    mlp_np = _np(MLP_DT)

    deg = np.bincount(recv, minlength=N_NODES)
    node_tile, node_slot = _balance_nodes(deg)

    # ---- edge packing into uniform [128, QT, F] per core ----
    t_e = node_tile[recv]
    slot_e = node_slot[recv]
    order_e = np.argsort(t_e, kind="stable")
    t_s = t_e[order_e]
    starts = np.searchsorted(t_s, np.arange(T_ALL + 1))
    rank = np.arange(N_EDGES, dtype=np.int64) - starts[t_s]
    chunk = rank >> 7
    lane = rank & 127
    core_s = t_s // NT
    q_s = (t_s % NT) * CPT + chunk          # chunk index within core
    slot_s = slot_e[order_e]
    ea_bf = None

    in_maps = []
    for c in range(N_CORES):
        m = core_s == c
        idx = lane[m] * QT + q_s[m]
        ebuf = np.zeros((128 * QT, F), edge_np)
        ebuf[idx] = edge_attr[order_e[m]].astype(edge_np)
        ebuf = ebuf.reshape(128, QT, F)
        rel = np.full(128 * QT, -1.0, np.float32)
        rel[idx] = slot_s[m].astype(np.float32)
        relbuf = rel.reshape(128, QT).astype(edge_np)
        in_maps.append({"edges": ebuf, "relT": relbuf})
    del ea_bf

    # ---- node features, permuted per core ----
    pos = (node_tile % NT) * TN + node_slot   # position within core
    core_n = node_tile // NT
    for c in range(N_CORES):
        mask = core_n == c
        xTc = np.zeros((128, NODES_PAD), mlp_np)
        xTc[:, pos[mask]] = x[mask].T.astype(mlp_np)
        in_maps[c]["xT"] = xTc

    # ---- shared (replicated) tensors ----
    b1_eff = b1 + (u[0] @ W1[256:256 + D_U])
    w1a = np.ascontiguousarray(W1[0:128]).astype(mlp_np)
    w1b = np.ascontiguousarray(W1[128:256]).astype(mlp_np)
    w2r = np.ascontiguousarray(
        W2.reshape(HC, 128, F).transpose(1, 0, 2).reshape(128, H)
    ).astype(mlp_np)
    b1T = np.ascontiguousarray(b1_eff.reshape(HC, 128).T).astype(np.float32)
    b2col = np.ascontiguousarray(b2.reshape(128, 1)).astype(np.float32)
    iota = np.tile(np.arange(TN, dtype=np.float32), (128, 1)).astype(edge_np)
    ident = np.eye(128, dtype=np.float32).astype(mlp_np)
    for c in range(N_CORES):
        in_maps[c].update({
            "iota": iota, "ident": ident, "w1a": w1a, "w1b": w1b,
            "w2r": w2r, "b1T": b1T, "b2c": b2col,
        })
    return in_maps, (core_n, pos)


_prog_cache = {}


def _get_program():
    key = (EDGE_DT, MLP_DT, OUT_DT)
    if key not in _prog_cache:
        _prog_cache[key] = build_program()
    return _prog_cache[key]


def run(inputs, trace=False, tmpdir=None):
    in_maps, (core_n, pos) = prepare_inputs(**inputs)
    nc = _get_program()
    res = bass_utils.run_bass_kernel_spmd(
        nc, in_maps, core_ids=list(range(N_CORES)), trace=trace,
        tmpdir=tmpdir)
    out = np.empty((N_NODES, F), np.float32)
    for c in range(N_CORES):
        mask = core_n == c
        yc = np.asarray(res.results[c]["y"], dtype=np.float32)  # [128, PAD]
        out[mask] = yc[:, pos[mask]].T
    return out, res


def kernel(**inputs) -> np.ndarray:
    out, _ = run(inputs, trace=False)
    return out


# revision 18
# speedup vs baseline: 1.0178x; 1.0178x over previous
"""GNN NodeBlock kernel for Trainium2, 8 NeuronCores (SPMD, no collectives).

Reference computation (N=50000 nodes, E=1600000 edges, F=128 features):
    recv_agg = segment_sum(edge_attr, edge_index[1], N)        # [N, 128]
    collected = concat([recv_agg, x, broadcast(u)], -1)        # [N, 272]
    out = relu(collected @ W1 + b1) @ W2 + b2                  # [N, 128]

Sharding: nodes are assigned to 8 cores x 196 tiles of 32 nodes by a
degree-balancing permutation (serpentine deal over degree-sorted nodes +
swap repair), so every tile owns <= 1024 edges and the edge buffer packs
into a uniform 8 chunks of 128 edges per tile with ~0.4% padding.  Edges
are bucketed by receiver-node ownership so the scatter-sum is local;
MLP weights are replicated; the u-term of layer 1 is folded into b1.

Device algorithm per core (196 tiles = 49 groups of 128 nodes):
  scatter: one DVE is_equal per supertile builds one-hot routing blocks
    onehot[e, q, n] = (rel[e, q] == n); for each 128-node group the PE
    runs 8 accumulation steps of 4 column-tiled matmuls (strip j =
    tile 4g+j, stationary = 32-col one-hot, moving = 128-col edge chunk)
    so all four 32x128 array strips compute concurrently;
    agg [n, f] is then PE-transposed to aggT [f, n].
  L1: h[h, n] = W1a[d, h]^T @ aggT[d, n] + W1b[d, h]^T @ xT[d, n],
    ReLU+bias on the scalar engine.
  L2: yT[f, n] = sum_hc W2r[h, f]^T @ hT[h, n]  (stationary = W2 chunk,
    streaming = hT), b2 added on the vector engine during PSUM evac.
  Output is stored feature-major [128, nodes]; the host transposes and
  un-permutes.
All matmul inputs are bf16 (fp32 PSUM accumulation); one-hots are exact.
"""

import numpy as np

from concourse import bacc, mybir, tile
from concourse import bass_utils
from concourse.bass_interp import get_hw_module

# ---------------- problem constants (hardcoded per spec) ----------------
N_NODES = 50000
N_EDGES = 1600000
F = 128           # edge/node feature dim
H = 1024          # hidden dim
HC = H // 128     # 8 hidden chunks
D_U = 16
N_CORES = 8
TN = 32                                # nodes per scatter tile
NT = 196                               # tiles per core
CPT = 8                                # chunks (of 128 edges) per tile
QT = NT * CPT                          # 1568 chunks per core
NODES_PAD = NT * TN                    # 6272 node slots per core
T_ALL = N_CORES * NT                   # 1568 tiles total
# supertile sizes (in tiles): small leading supers shorten the pipeline
# ramp; 16-tile (512-node) supers amortize matmul/LDW/ACT overheads
SUPERS = [4, 4, 8] + [16] * 11 + [4]
assert sum(SUPERS) == NT

EDGE_DT = mybir.dt.bfloat16
MLP_DT = mybir.dt.bfloat16
OUT_DT = mybir.dt.bfloat16             # on-device output store dtype

_np = mybir.dt.np  # mybir dtype -> numpy dtype


# ---------------- device program ----------------

def build_program():
    f32 = mybir.dt.float32

    nc = bacc.Bacc("TRN2", target_bir_lowering=False, debug=False,
                   num_devices=N_CORES)

    edges = nc.dram_tensor("edges", [128, QT, F], EDGE_DT,
                           kind="ExternalInput").ap()
    relT = nc.dram_tensor("relT", [128, QT], EDGE_DT,
                          kind="ExternalInput").ap()
    iota = nc.dram_tensor("iota", [128, TN], EDGE_DT,
                          kind="ExternalInput").ap()
    ident = nc.dram_tensor("ident", [128, 128], MLP_DT,
                           kind="ExternalInput").ap()
    xT = nc.dram_tensor("xT", [128, NODES_PAD], MLP_DT,
                        kind="ExternalInput").ap()
    w1a = nc.dram_tensor("w1a", [128, H], MLP_DT, kind="ExternalInput").ap()
    w1b = nc.dram_tensor("w1b", [128, H], MLP_DT, kind="ExternalInput").ap()
    w2r = nc.dram_tensor("w2r", [128, H], MLP_DT, kind="ExternalInput").ap()
    b1T = nc.dram_tensor("b1T", [128, HC], f32, kind="ExternalInput").ap()
    b2c = nc.dram_tensor("b2c", [128, 1], f32, kind="ExternalInput").ap()
    y = nc.dram_tensor("y", [128, NODES_PAD], OUT_DT,
                       kind="ExternalOutput").ap()

    with tile.TileContext(nc) as tc:
        with (
            tc.tile_pool(name="const", bufs=1) as cpool,
            tc.tile_pool(name="edge", bufs=6) as epool,
            tc.tile_pool(name="oh", bufs=6) as ohpool,
            tc.tile_pool(name="nf", bufs=9) as nfpool,
            tc.tile_pool(name="aggT", bufs=2) as aggpool,
            tc.tile_pool(name="h", bufs=18) as hpool,
            tc.tile_pool(name="out", bufs=2) as outpool,
            tc.tile_pool(name="ps_agg", bufs=2, space="PSUM") as ps_agg,
            tc.tile_pool(name="ps_tr", bufs=2, space="PSUM") as ps_tr,
            tc.tile_pool(name="ps_h", bufs=2, space="PSUM") as ps_h,
            tc.tile_pool(name="ps_o", bufs=2, space="PSUM") as ps_o,
        ):
            # relT/iota gate the first one-hot: load them on the sync ring
            # AHEAD of the edge stream; bulk constants go on the scalar ring
            relT_sb = cpool.tile([128, QT], EDGE_DT, tag="relT")
            nc.sync.dma_start(relT_sb[:], relT[:])
            iota_sb = cpool.tile([128, TN], EDGE_DT, tag="iota")
            nc.sync.dma_start(iota_sb[:], iota[:])
            ident_sb = cpool.tile([128, 128], MLP_DT, tag="ident")
            nc.scalar.dma_start(ident_sb[:], ident[:])
            xT_sb = cpool.tile([128, NODES_PAD], MLP_DT, tag="xT")
            nc.scalar.dma_start(xT_sb[:], xT[:])
            w1a_sb = cpool.tile([128, H], MLP_DT, tag="w1a")
            nc.scalar.dma_start(w1a_sb[:], w1a[:])
            w1b_sb = cpool.tile([128, H], MLP_DT, tag="w1b")
            nc.scalar.dma_start(w1b_sb[:], w1b[:])
            w2r_sb = cpool.tile([128, H], MLP_DT, tag="w2r")
            nc.scalar.dma_start(w2r_sb[:], w2r[:])
            b1T_sb = cpool.tile([128, HC], f32, tag="b1T")
            nc.scalar.dma_start(b1T_sb[:], b1T[:])
            b2c_sb = cpool.tile([128, 1], f32, tag="b2c")
            nc.scalar.dma_start(b2c_sb[:], b2c[:])

            def quad(ps_out, wsb, col0, rhs_ap, start, stop):
                """One 128-col matmul issued as 4 concurrent 32-col strips
                so LDWEIGHTS hides in per-subarray concurrency."""
                for j in range(4):
                    nc.tensor.matmul(
                        ps_out[j * 32:(j + 1) * 32, :],
                        lhsT=wsb[:, col0 + j * 32:col0 + (j + 1) * 32],
                        rhs=rhs_ap,
                        start=start, stop=stop,
                        skip_group_check=True,
                        tile_position=(0, j * 32),
                    )

            def flush_l2(prev, o_ps):
                """Evacuate + store the finished previous supertile's yT."""
                t0p, nnp, _ = prev
                y_sb = outpool.tile([128, nnp], OUT_DT, tag="y")
                b2_bc = b2c_sb[:].broadcast_to([128, nnp])
                nc.vector.tensor_tensor(out=y_sb[:], in0=o_ps[:], in1=b2_bc,
                                        op=mybir.AluOpType.add)
                nc.scalar.dma_start(y[:, t0p * TN:t0p * TN + nnp], y_sb[:])

            def do_scatter(t0s, nts):
                """Scatter a supertile in 128-node groups: per-group edge
                DMA (1 MB) + one-hot + 8 accumulation steps of 4
                column-tiled strips.  Fine granularity keeps PE idle
                slices under the HAM re-throttle window."""
                nn = nts * TN
                q0 = t0s * CPT
                agg_nfs = []
                for g in range(nn // 128):
                    gq0 = q0 + g * 4 * CPT     # 32 chunks per group
                    e_g = epool.tile([128, 4 * CPT, F], EDGE_DT, tag="e")
                    nc.sync.dma_start(e_g[:], edges[:, gq0:gq0 + 4 * CPT])
                    oh = ohpool.tile([128, 4 * CPT, TN], EDGE_DT, tag="oh")
                    rel_bc = relT_sb[:, gq0:gq0 + 4 * CPT].rearrange(
                        "p (c u) -> p c u", u=1).broadcast_to(
                            [128, 4 * CPT, TN])
                    iota_bc = iota_sb[:].rearrange(
                        "p (u n) -> p u n", u=1).broadcast_to(
                            [128, 4 * CPT, TN])
                    nc.vector.tensor_tensor(out=oh[:], in0=iota_bc,
                                            in1=rel_bc,
                                            op=mybir.AluOpType.is_equal)
                    agg_ps = ps_agg.tile([128, 128], f32, tag="agg")
                    for c in range(CPT):
                        for j in range(4):
                            q = j * CPT + c
                            nc.tensor.matmul(
                                agg_ps[j * TN:(j + 1) * TN, :],
                                lhsT=oh[:, q, :],
                                rhs=e_g[:, q, :],
                                start=(c == 0),
                                stop=(c == CPT - 1),
                                skip_group_check=True,
                                tile_position=(0, j * TN),
                            )
                    agg_nf = nfpool.tile([128, 128], MLP_DT, tag="nf")
                    nc.scalar.copy(agg_nf[:], agg_ps[:])
                    agg_nfs.append(agg_nf)
                return (t0s, nn, agg_nfs)

            def do_tr_l1(sc, l2prev):
                """Transpose + L1 for a scattered supertile (one super
                behind the scatter), interleaved with L2 of the super
                before that — every PE op here depends only on data
                finished at least a supertile ago, so the in-order PE
                queue never stalls on the scalar/vector engines."""
                t0s, nn, agg_nfs = sc
                aggT = aggpool.tile([128, nn], MLP_DT, tag="aggT")
                for g, agg_nf in enumerate(agg_nfs):
                    tr_ps = ps_tr.tile([128, 128], MLP_DT, tag="tr")
                    nc.tensor.transpose(tr_ps[:], agg_nf[:], ident_sb[:])
                    nc.vector.tensor_copy(aggT[:, g * 128:(g + 1) * 128],
                                          tr_ps[:])
                o_ps = None
                if l2prev is not None:
                    o_ps = ps_o.tile([128, l2prev[1]], f32, tag="ops")
                hTs = []
                for hc in range(HC):
                    h_ps = ps_h.tile([128, nn], f32, tag="h")
                    quad(h_ps, w1a_sb, hc * 128, aggT[:],
                         start=True, stop=False)
                    quad(h_ps, w1b_sb, hc * 128,
                         xT_sb[:, t0s * TN:t0s * TN + nn],
                         start=False, stop=True)
                    if l2prev is not None:
                        quad(o_ps, w2r_sb, hc * 128, l2prev[2][hc][:],
                             start=(hc == 0), stop=(hc == HC - 1))
                    hT = hpool.tile([128, nn], MLP_DT, tag="hT")
                    nc.scalar.activation(hT[:], h_ps[:],
                                         mybir.ActivationFunctionType.Relu,
                                         bias=b1T_sb[:, hc:hc + 1], scale=1.0)
                    hTs.append(hT)
                if l2prev is not None:
                    flush_l2(l2prev, o_ps)
                return (t0s, nn, hTs)

            t0 = 0
            sc_prev = None       # scattered, awaiting transpose+L1
            l1_prev = None       # L1 done, awaiting L2
            for s, nts in enumerate(SUPERS):
                # tr/L1 of the previous super is emitted FIRST so the DVE
                # queue serves its transpose-evacuation copies before the
                # next super's one-hot builds
                if sc_prev is not None:
                    l1_prev = do_tr_l1(sc_prev, l1_prev)
                sc_prev = do_scatter(t0, nts)
                t0 += nts
            l1_prev = do_tr_l1(sc_prev, l1_prev)
            # epilogue: layer 2 of the last supertile
            o_ps = ps_o.tile([128, l1_prev[1]], f32, tag="ops")
            for hc in range(HC):
                quad(o_ps, w2r_sb, hc * 128, l1_prev[2][hc][:],
                     start=(hc == 0), stop=(hc == HC - 1))
            flush_l2(l1_prev, o_ps)

    nc.compile()
    nc.m = get_hw_module(nc.m)
    return nc


# ---------------- host-side sharding / packing ----------------

def _balance_nodes(deg):
    """Assign nodes to (tile, slot) so every tile of 32 nodes owns
    <= CPT*128 edges.  Serpentine deal over degree-sorted nodes, then a
    swap repair for the handful of tiles that land 1-2 edges over."""
    n = deg.shape[0]
    order = np.argsort(-deg, kind="stable")
    node_tile = np.empty(n, np.int64)
    node_slot = np.empty(n, np.int64)
    rounds = (n + T_ALL - 1) // T_ALL
    for r in range(rounds):
        blk = order[r * T_ALL:(r + 1) * T_ALL]
        tiles = np.arange(len(blk))
        if r % 2 == 1:
            tiles = T_ALL - 1 - tiles
        node_tile[blk] = tiles
        node_slot[blk] = r
    cap = CPT * 128
    loads = np.bincount(node_tile, weights=deg.astype(np.float64),
                        minlength=T_ALL).astype(np.int64)
    if loads.max() > cap:
        # per-tile node lists for the repair pass
        by_tile = [[] for _ in range(T_ALL)]
        for nd in range(n):
            by_tile[node_tile[nd]].append(nd)
        lo_tiles = list(np.argsort(loads)[:4 * (loads > cap).sum() + 8])
        for t_hi in np.where(loads > cap)[0]:
            while loads[t_hi] > cap:
                need = loads[t_hi] - cap
                swapped = False
                for a in sorted(by_tile[t_hi], key=lambda v: -deg[v]):
                    for t_lo in lo_tiles:
                        room = cap - loads[t_lo]
                        for b in by_tile[t_lo]:
                            d = deg[a] - deg[b]
                            if need <= d <= room:
                                sa, sb = node_slot[a], node_slot[b]
                                node_tile[a], node_tile[b] = t_lo, t_hi
                                node_slot[a], node_slot[b] = sb, sa
                                by_tile[t_hi].remove(a)
                                by_tile[t_lo].remove(b)
                                by_tile[t_lo].append(a)
                                by_tile[t_hi].append(b)
                                loads[t_hi] -= d
                                loads[t_lo] += d
                                swapped = True
                                break
                        if swapped:
                            break
                    if swapped:
                        break
                assert swapped, "balance repair failed"
    assert loads.max() <= cap
    return node_tile, node_slot


def prepare_inputs(x, edge_attr, u, W1, b1, W2, b2, edge_index):
    x = np.asarray(x, dtype=np.float32)
    edge_attr = np.asarray(edge_attr, dtype=np.float32)
    u = np.asarray(u, dtype=np.float32)
    W1 = np.asarray(W1, dtype=np.float32)
    b1 = np.asarray(b1, dtype=np.float32)
    W2 = np.asarray(W2, dtype=np.float32)
    b2 = np.asarray(b2, dtype=np.float32)
    recv = np.asarray(edge_index)[1].astype(np.int64)

    edge_np = _np(EDGE_DT)
    mlp_np = _np(MLP_DT)

    deg = np.bincount(recv, minlength=N_NODES)
    node_tile, node_slot = _balance_nodes(deg)

    # ---- edge packing into uniform [128, QT, F] per core ----
    t_e = node_tile[recv]
    slot_e = node_slot[recv]
    order_e = np.argsort(t_e, kind="stable")
    t_s = t_e[order_e]
    starts = np.searchsorted(t_s, np.arange(T_ALL + 1))
    rank = np.arange(N_EDGES, dtype=np.int64) - starts[t_s]
    chunk = rank >> 7
    lane = rank & 127
    core_s = t_s // NT
    q_s = (t_s % NT) * CPT + chunk          # chunk index within core
    slot_s = slot_e[order_e]
    ea_bf = None

    in_maps = []
    for c in range(N_CORES):
        m = core_s == c
        idx = lane[m] * QT + q_s[m]
        ebuf = np.zeros((128 * QT, F), edge_np)
        ebuf[idx] = edge_attr[order_e[m]].astype(edge_np)
        ebuf = ebuf.reshape(128, QT, F)
        rel = np.full(128 * QT, -1.0, np.float32)
        rel[idx] = slot_s[m].astype(np.float32)
        relbuf = rel.reshape(128, QT).astype(edge_np)
        in_maps.append({"edges": ebuf, "relT": relbuf})
    del ea_bf

    # ---- node features, permuted per core ----
    pos = (node_tile % NT) * TN + node_slot   # position within core
    core_n = node_tile // NT
    for c in range(N_CORES):
        mask = core_n == c
        xTc = np.zeros((128, NODES_PAD), mlp_np)
        xTc[:, pos[mask]] = x[mask].T.astype(mlp_np)
        in_maps[c]["xT"] = xTc

    # ---- shared (replicated) tensors ----
    b1_eff = b1 + (u[0] @ W1[256:256 + D_U])
    w1a = np.ascontiguousarray(W1[0:128]).astype(mlp_np)
    w1b = np.ascontiguousarray(W1[128:256]).astype(mlp_np)
    w2r = np.ascontiguousarray(
        W2.reshape(HC, 128, F).transpose(1, 0, 2).reshape(128, H)
    ).astype(mlp_np)
    b1T = np.ascontiguousarray(b1_eff.reshape(HC, 128).T).astype(np.float32)
    b2col = np.ascontiguousarray(b2.reshape(128, 1)).astype(np.float32)
    iota = np.tile(np.arange(TN, dtype=np.float32), (128, 1)).astype(edge_np)
    ident = np.eye(128, dtype=np.float32).astype(mlp_np)
    for c in range(N_CORES):
        in_maps[c].update({
            "iota": iota, "ident": ident, "w1a": w1a, "w1b": w1b,
            "w2r": w2r, "b1T": b1T, "b2c": b2col,
        })
    return in_maps, (core_n, pos)


_prog_cache = {}


def _get_program():
    key = (EDGE_DT, MLP_DT, OUT_DT)
    if key not in _prog_cache:
        _prog_cache[key] = build_program()
    return _prog_cache[key]


def run(inputs, trace=False, tmpdir=None):
    in_maps, (core_n, pos) = prepare_inputs(**inputs)
    nc = _get_program()
    res = bass_utils.run_bass_kernel_spmd(
        nc, in_maps, core_ids=list(range(N_CORES)), trace=trace,
        tmpdir=tmpdir)
    out = np.empty((N_NODES, F), np.float32)
    for c in range(N_CORES):
        mask = core_n == c
        yc = np.asarray(res.results[c]["y"], dtype=np.float32)  # [128, PAD]
        out[mask] = yc[:, pos[mask]].T
    return out, res


def kernel(**inputs) -> np.ndarray:
    out, _ = run(inputs, trace=False)
    return out
